# revision 6
# baseline (speedup 1.0000x reference)
"""Trainium2 Bass kernel: causal multi-head attention block (QKV proj + RoPE +
causal softmax attention + output proj), head-sharded across 8 NeuronCores.

Sharding: tensor-parallel over heads. Core c owns heads (2c, 2c+1): it computes
q/k/v projections for its 256 qkv dims from the full x, runs causal attention
for its 2 heads, and produces a partial output projection y_c = out_c @ Wo_c^T.
The host sums the 8 partials (the all-reduce).

Self-contained: hardcodes B=2, L=2048, H=2048, NH=16, HD=128.
"""

import math
from contextlib import ExitStack

import numpy as np

import concourse.bass as bass
import concourse.tile as tile
from concourse import bacc, mybir
from concourse.bass_utils import run_bass_kernel_spmd

# problem constants
B, L, H, NH, HD = 2, 2048, 2048, 16, 128
ROPE_THETA = 10000.0
N_CORES = 8
NH_LOC = NH // N_CORES          # heads per core = 2
QKV_LOC = 3 * NH_LOC * HD       # 768 qkv dims per core
D_LOC = NH_LOC * HD             # 256
BL = B * L                      # 4096 tokens
P = 128
KC = H // P                     # 16 contraction chunks
BLK = 512                       # token block in projection phase
NBLK = BL // BLK                # 8
QS = 512                        # q slice in attention phase
NQS = L // QS                   # 4
KT = L // P                     # 16 k tiles per batch
NBH = B * NH_LOC                # 4 (batch, head) pairs per core

F32 = mybir.dt.float32
F32R = mybir.dt.float32r
EXP = mybir.ActivationFunctionType.Exp
NEG = -30000.0                  # additive mask; exp(NEG + s) == 0.0 in fp32


def _build():
    nc = bacc.Bacc("TRN2", target_bir_lowering=False, debug=False,
                   num_devices=N_CORES)

    xT = nc.dram_tensor("xT", [H, BL], F32R, kind="ExternalInput").ap()
    wT = nc.dram_tensor("wT", [H, QKV_LOC], F32R, kind="ExternalInput").ap()
    woT = nc.dram_tensor("woT", [D_LOC, H], F32R, kind="ExternalInput").ap()
    cosT = nc.dram_tensor("cosT", [HD, L], F32, kind="ExternalInput").ap()
    sinTs = nc.dram_tensor("sinTs", [HD, L], F32, kind="ExternalInput").ap()
    mask4 = nc.dram_tensor("mask4", [QS // P, P, QS], F32,
                           kind="ExternalInput").ap()
    ones_in = nc.dram_tensor("ones", [P, P], F32R, kind="ExternalInput").ap()
    y = nc.dram_tensor("y", [BL, H], F32, kind="ExternalOutput").ap()

    qT_d = nc.dram_tensor("qT_d", [NBH, HD, L], F32R).ap()
    kT_d = nc.dram_tensor("kT_d", [NBH, HD, L], F32R).ap()
    vN_d = nc.dram_tensor("vN_d", [NBH, L, HD], F32R).ap()

    with tile.TileContext(nc) as tc, ExitStack() as ctx:
        g = ctx.enter_context(tc.tile_pool(name="g", bufs=1))
        # attention outputs, transposed (hd, bh, q); resident across phases
        outT = g.tile([P, NBH, L], F32R)

        # ---------------- phase 1: QKV projection + RoPE ----------------
        with tc.tile_pool(name="p1w", bufs=1) as p1w, \
             tc.tile_pool(name="p1x", bufs=2) as p1x, \
             tc.tile_pool(name="p1t", bufs=3) as p1t, \
             tc.tile_pool(name="ps1", bufs=2, space="PSUM") as ps1, \
             tc.tile_pool(name="ps1v", bufs=2, space="PSUM") as ps1v:
            wt = p1w.tile([P, KC, QKV_LOC], F32R)
            for kc in range(KC):
                nc.sync.dma_start(wt[:, kc, :], wT[kc * P:(kc + 1) * P, :])
            cost = p1w.tile([P, L], F32)
            nc.sync.dma_start(cost[:], cosT[:])
            sints = p1w.tile([P, L], F32)
            nc.sync.dma_start(sints[:], sinTs[:])

            for blk in range(NBLK):
                b, lo = divmod(blk, NBLK // B)
                lo *= BLK                      # token offset within batch
                col = blk * BLK                # column in xT
                xb = p1x.tile([P, KC, BLK], F32R, name="xb")
                for kc in range(KC):
                    nc.sync.dma_start(xb[:, kc, :],
                                      xT[kc * P:(kc + 1) * P, col:col + BLK])
                # q/k dim tiles (128 each): 0,1 = q heads; 2,3 = k heads
                for dt_i in range(4):
                    qk, hh = divmod(dt_i, 2)
                    wcol = dt_i * P
                    psum = ps1.tile([P, BLK], F32, name="qkps")
                    for kc in range(KC):
                        nc.tensor.matmul(
                            psum[:],
                            lhsT=wt[:, kc, wcol:wcol + P],
                            rhs=xb[:, kc, :],
                            start=(kc == 0), stop=(kc == KC - 1))
                    # rope: out = x*cos + rot(x)*sin_signed, rot = partition
                    # swap halves (via SBUF->SBUF DMA; sign baked into sinTs)
                    qc = p1t.tile([P, BLK], F32, name="qc")
                    nc.vector.tensor_copy(qc[:], psum[:])
                    qsw = p1t.tile([P, BLK], F32, name="qsw")
                    nc.sync.dma_start(qsw[0:64, :], qc[64:128, :])
                    nc.sync.dma_start(qsw[64:128, :], qc[0:64, :])
                    t1 = p1t.tile([P, BLK], F32, name="t1")
                    nc.vector.tensor_mul(t1[:], qc[:], cost[:, lo:lo + BLK])
                    t2 = p1t.tile([P, BLK], F32, name="t2")
                    nc.vector.tensor_mul(t2[:], qsw[:], sints[:, lo:lo + BLK])
                    qr = p1t.tile([P, BLK], F32R, name="qr")
                    nc.vector.tensor_add(qr[:], t1[:], t2[:])
                    dst = qT_d if qk == 0 else kT_d
                    nc.sync.dma_start(dst[b * NH_LOC + hh, :, lo:lo + BLK],
                                      qr[:])
                # v in natural (token, hd) layout, both heads at once
                for tt in range(BLK // P):
                    psv = ps1v.tile([P, D_LOC], F32, name="vps")
                    for kc in range(KC):
                        nc.tensor.matmul(
                            psv[:],
                            lhsT=xb[:, kc, tt * P:(tt + 1) * P],
                            rhs=wt[:, kc, 2 * D_LOC:3 * D_LOC],
                            start=(kc == 0), stop=(kc == KC - 1))
                    vsb = p1t.tile([P, D_LOC], F32R, name="vsb")
                    nc.vector.tensor_copy(vsb[:], psv[:])
                    tok = lo + tt * P
                    for hh in range(NH_LOC):
                        nc.sync.dma_start(
                            vN_d[b * NH_LOC + hh, tok:tok + P, :],
                            vsb[:, hh * HD:(hh + 1) * HD])

        # ---------------- phase 2: causal attention ----------------
        with tc.tile_pool(name="p2c", bufs=1) as p2c, \
             tc.tile_pool(name="p2b", bufs=2) as p2b, \
             tc.tile_pool(name="p2e", bufs=4) as p2e, \
             tc.tile_pool(name="p2t", bufs=3) as p2t, \
             tc.tile_pool(name="ps2s", bufs=2, space="PSUM") as ps2s, \
             tc.tile_pool(name="ps2o", bufs=2, space="PSUM") as ps2o, \
             tc.tile_pool(name="ps2d", bufs=2, space="PSUM") as ps2d:
            maskt = p2c.tile([P, QS // P, QS], F32)
            nc.sync.dma_start(maskt[:], mask4.rearrange("m p q -> p m q"))
            ones = p2c.tile([P, P], F32R)
            nc.sync.dma_start(ones[:], ones_in[:])

            for bh in range(NBH):
                qt = p2b.tile([P, L], F32R, name="qt")
                nc.sync.dma_start(qt[:], qT_d[bh])
                kt = p2b.tile([P, L], F32R, name="kt")
                nc.sync.dma_start(kt[:], kT_d[bh])
                vn = p2b.tile([P, KT, HD], F32R, name="vn")
                nc.sync.dma_start(vn[:],
                                  vN_d[bh].rearrange("(n p) d -> p n d", p=P))
                for qs_i in range(NQS):
                    qs = qs_i * QS
                    nkt = (qs + QS) // P
                    po = ps2o.tile([P, QS], F32, name="po")
                    pd = ps2d.tile([P, QS], F32, name="pd")
                    for k_i in range(nkt):
                        psc = ps2s.tile([P, QS], F32, name="psc")
                        nc.tensor.matmul(
                            psc[:],
                            lhsT=kt[:, k_i * P:(k_i + 1) * P],
                            rhs=qt[:, qs:qs + QS],
                            start=True, stop=True)
                        et = p2e.tile([P, QS], F32R, name="et")
                        d = k_i * P - qs
                        if d >= 0:          # tile overlaps the diagonal
                            sm = p2t.tile([P, QS], F32, name="sm")
                            nc.vector.tensor_add(sm[:], psc[:],
                                                 maskt[:, d // P, :])
                            nc.scalar.activation(et[:], sm[:], EXP)
                        else:
                            nc.scalar.activation(et[:], psc[:], EXP)
                        etr = et[:]
                        nc.tensor.matmul(po[:],
                                         lhsT=vn[:, k_i, :],
                                         rhs=etr, start=(k_i == 0),
                                         stop=(k_i == nkt - 1))
                        nc.tensor.matmul(pd[:], lhsT=ones[:],
                                         rhs=etr, start=(k_i == 0),
                                         stop=(k_i == nkt - 1))
                    rec = p2t.tile([P, QS], F32, name="rec")
                    nc.vector.reciprocal(rec[:], pd[:])
                    nc.vector.tensor_mul(outT[:, bh, qs:qs + QS], po[:],
                                         rec[:])

        # ---------------- phase 3: output projection (partial) ----------------
        with tc.tile_pool(name="p3w", bufs=1) as p3w, \
             tc.tile_pool(name="p3t", bufs=3) as p3t, \
             tc.tile_pool(name="ps3", bufs=2, space="PSUM") as ps3:
            wo = p3w.tile([P, NH_LOC, H], F32R)
            for hh in range(NH_LOC):
                nc.sync.dma_start(wo[:, hh, :], woT[hh * P:(hh + 1) * P, :])
            for b in range(B):
                for tt in range(L // P):
                    for oc in range(H // 512):
                        py_ = ps3.tile([P, 512], F32, name="py")
                        for hh in range(NH_LOC):
                            nc.tensor.matmul(
                                py_[:],
                                lhsT=outT[:, b * NH_LOC + hh,
                                          tt * P:(tt + 1) * P],
                                rhs=wo[:, hh, oc * 512:(oc + 1) * 512]
                                    ,
                                start=(hh == 0), stop=(hh == NH_LOC - 1))
                        yt = p3t.tile([P, 512], F32, name="yt")
                        nc.vector.tensor_copy(yt[:], py_[:])
                        nc.sync.dma_start(
                            y[b * L + tt * P: b * L + (tt + 1) * P,
                              oc * 512:(oc + 1) * 512], yt[:])

    nc.compile()
    return nc


_NC = None


def _get_nc():
    global _NC
    if _NC is None:
        _NC = _build()
    return _NC


def _host_inputs(x, Wqkv, Wo):
    """Shard + preprocess the full inputs into per-core input maps."""
    x = np.asarray(x, dtype=np.float32)
    Wqkv = np.asarray(Wqkv, dtype=np.float32)
    Wo = np.asarray(Wo, dtype=np.float32)

    xT = np.ascontiguousarray(x.reshape(BL, H).T)          # (H, BL)

    # rope tables, transposed to (hd, L); sign baked into sin for the
    # rotate-half swap; 1/sqrt(HD) score scale folded into Wq.
    inv_freq = 1.0 / (ROPE_THETA ** (np.arange(0, HD, 2, dtype=np.float32)
                                     / HD))
    t = np.arange(L, dtype=np.float32)
    freqs = np.outer(t, inv_freq).astype(np.float32)       # (L, 64)
    emb = np.concatenate([freqs, freqs], axis=-1)          # (L, HD)
    cosT = np.ascontiguousarray(np.cos(emb).T.astype(np.float32))
    sinT = np.sin(emb).T.astype(np.float32)
    sinTs = np.concatenate([-sinT[:64], sinT[64:]], axis=0)
    sinTs = np.ascontiguousarray(sinTs)

    # additive causal masks for the 4 diagonal positions of a (128k, 512q)
    # score tile whose q-slice starts at k_tile_start - d
    mask4 = np.zeros((QS // P, P, QS), dtype=np.float32)
    kk = np.arange(P)[:, None]
    qq = np.arange(QS)[None, :]
    for i in range(QS // P):
        d = i * P
        mask4[i] = np.where(qq >= d + kk, 0.0, NEG)

    scale = np.float32(1.0 / math.sqrt(HD))
    in_maps = []
    for c in range(N_CORES):
        r0 = c * D_LOC
        wq = Wqkv[r0:r0 + D_LOC] * scale
        wk = Wqkv[H + r0:H + r0 + D_LOC]
        wv = Wqkv[2 * H + r0:2 * H + r0 + D_LOC]
        wT_c = np.ascontiguousarray(
            np.concatenate([wq, wk, wv], axis=0).T)        # (H, 768)
        woT_c = np.ascontiguousarray(Wo[:, r0:r0 + D_LOC].T)  # (256, H)
        in_maps.append({
            "xT": xT, "wT": wT_c, "woT": woT_c,
            "cosT": cosT, "sinTs": sinTs, "mask4": mask4,
            "ones": np.ones((P, P), dtype=np.float32),
        })
    return in_maps


def kernel(x, Wqkv, Wo):
    nc = _get_nc()
    in_maps = _host_inputs(x, Wqkv, Wo)
    res = run_bass_kernel_spmd(nc, in_maps, list(range(N_CORES)))
    y = res.results[0]["y"].astype(np.float64)
    for c in range(1, N_CORES):
        y += res.results[c]["y"]
    return y.astype(np.float32).reshape(B, L, H)


# revision 7
# speedup vs baseline: 1.4186x; 1.4186x over previous
"""V3: v2 + DMA batching (HWDGE issue overhead was the phase-1 bottleneck).

- x block load: one DMA via (kc p) t -> p kc t rearrange (was 16).
- weight load: one DMA (was 16).
- rope rotate-half swap batched across the 4 q/k dim-tiles (2 DMAs/block).
- q spill + v spill: one multi-dim DMA per block each.
- y written in (128, 2048) row blocks (one DMA per token tile), with the
  PSUM->SBUF copies on the scalar engine (DVE was saturating).
"""

import math
from contextlib import ExitStack

import numpy as np

import concourse.bass as bass
import concourse.tile as tile
from concourse import bacc, mybir
from concourse.bass_utils import run_bass_kernel_spmd

B, L, H, NH, HD = 2, 2048, 2048, 16, 128
ROPE_THETA = 10000.0
N_CORES = 8
NH_LOC = NH // N_CORES          # 2
QKV_LOC = 3 * NH_LOC * HD       # 768
D_LOC = NH_LOC * HD             # 256
BL = B * L
P = 128
KC = H // P                     # 16
BLK = 256
NBLK = BL // BLK                # 16
BLK_PER_B = NBLK // B           # 8
QS = 512
NQS = L // QS
KT = L // P
NBH = B * NH_LOC                # 4

F32 = mybir.dt.float32
F32R = mybir.dt.float32r
EXP = mybir.ActivationFunctionType.Exp
NEG = -30000.0


def _build():
    nc = bacc.Bacc("TRN2", target_bir_lowering=False, debug=False,
                   num_devices=N_CORES)

    xT = nc.dram_tensor("xT", [H, BL], F32R, kind="ExternalInput").ap()
    wT = nc.dram_tensor("wT", [H, QKV_LOC], F32R, kind="ExternalInput").ap()
    woT = nc.dram_tensor("woT", [D_LOC, H], F32R, kind="ExternalInput").ap()
    cosT = nc.dram_tensor("cosT", [HD, L], F32, kind="ExternalInput").ap()
    sinTs = nc.dram_tensor("sinTs", [HD, L], F32, kind="ExternalInput").ap()
    tri = nc.dram_tensor("tri", [P, P], F32, kind="ExternalInput").ap()
    ones_in = nc.dram_tensor("ones", [P, P], F32R, kind="ExternalInput").ap()
    y = nc.dram_tensor("y", [BL, H], F32, kind="ExternalOutput").ap()

    qT_d = nc.dram_tensor("qT_d", [NBH, HD, L], F32R).ap()
    vN_d = nc.dram_tensor("vN_d", [NBH, L, HD], F32R).ap()
    oT_d = nc.dram_tensor("oT_d", [NBH, HD, L], F32R).ap()

    with tile.TileContext(nc) as tc, ExitStack() as ctx:
        g = ctx.enter_context(tc.tile_pool(name="g", bufs=1))
        kt_all = g.tile([P, NBH, L], F32R)      # resident rope'd k-cache

        p2c = ctx.enter_context(tc.tile_pool(name="p2c", bufs=1))
        p2q = ctx.enter_context(tc.tile_pool(name="p2q", bufs=2))
        p2v = ctx.enter_context(tc.tile_pool(name="p2v", bufs=1))
        p2e = ctx.enter_context(tc.tile_pool(name="p2e", bufs=6))
        p2t = ctx.enter_context(tc.tile_pool(name="p2t", bufs=2))
        ps2s = ctx.enter_context(tc.tile_pool(name="ps2s", bufs=2, space="PSUM"))
        ps2o = ctx.enter_context(tc.tile_pool(name="ps2o", bufs=1, space="PSUM"))
        ps2d = ctx.enter_context(tc.tile_pool(name="ps2d", bufs=1, space="PSUM"))

        # ---------------- phase 1: QKV projection + RoPE ----------------
        with tc.tile_pool(name="p1w", bufs=1) as p1w, \
             tc.tile_pool(name="p1x", bufs=2) as p1x, \
             tc.tile_pool(name="p1t", bufs=2) as p1t, \
             tc.tile_pool(name="ps1", bufs=2, space="PSUM") as ps1, \
             tc.tile_pool(name="ps1v", bufs=2, space="PSUM") as ps1v:
            wt = p1w.tile([P, KC, QKV_LOC], F32R)
            nc.sync.dma_start(wt[:, 0:2, :],
                              wT[0:2 * P, :].rearrange("(n p) d -> p n d", p=P))
            xb0 = p1x.tile([P, KC, BLK], F32R, name="xb")
            nc.sync.dma_start(xb0[:],
                              xT[:, 0:BLK].rearrange("(n p) t -> p n t", p=P))
            for kq in range(2, KC, 7):
                hi = min(kq + 7, KC)
                nc.sync.dma_start(
                    wt[:, kq:hi, :],
                    wT[kq * P:hi * P, :].rearrange("(n p) d -> p n d", p=P))
            cost = p1w.tile([P, L], F32)
            sints = p1w.tile([P, L], F32)
            for ch in range(4):
                sl = slice(ch * 512, (ch + 1) * 512)
                nc.sync.dma_start(cost[:, sl], cosT[:, sl])
                nc.sync.dma_start(sints[:, sl], sinTs[:, sl])

            for blk in range(NBLK):
                b, lo = divmod(blk, BLK_PER_B)
                lo *= BLK
                col = blk * BLK
                if blk == 0:
                    xb = xb0
                else:
                    xb = p1x.tile([P, KC, BLK], F32R, name="xb")
                    nc.sync.dma_start(
                        xb[:], xT[:, col:col + BLK]
                        .rearrange("(n p) t -> p n t", p=P))

                qc = p1t.tile([P, 4, BLK], F32, name="qc")
                for dt_i in range(4):     # 0,1 = q heads; 2,3 = k heads
                    psum = ps1.tile([P, BLK], F32, name="qkps")
                    for kc in range(KC):
                        nc.tensor.matmul(
                            psum[:], lhsT=wt[:, kc, dt_i * P:dt_i * P + P],
                            rhs=xb[:, kc, :],
                            start=(kc == 0), stop=(kc == KC - 1))
                    nc.vector.tensor_copy(qc[:, dt_i, :], psum[:])
                # batched rotate-half swap for all 4 dim-tiles
                qsw = p1t.tile([P, 4, BLK], F32, name="qsw")
                nc.sync.dma_start(qsw[0:64, :, :], qc[64:128, :, :])
                nc.sync.dma_start(qsw[64:128, :, :], qc[0:64, :, :])

                qr = p1t.tile([P, NH_LOC, BLK], F32R, name="qr")
                for dt_i in range(4):
                    qk, hh = divmod(dt_i, 2)
                    bh = b * NH_LOC + hh
                    t1 = p1t.tile([P, BLK], F32, name="t1")
                    nc.vector.tensor_mul(t1[:], qc[:, dt_i, :],
                                         cost[:, lo:lo + BLK])
                    t2 = p1t.tile([P, BLK], F32, name="t2")
                    nc.vector.tensor_mul(t2[:], qsw[:, dt_i, :],
                                         sints[:, lo:lo + BLK])
                    if qk == 0:
                        nc.vector.tensor_add(qr[:, hh, :], t1[:], t2[:])
                    else:
                        nc.vector.tensor_add(kt_all[:, bh, lo:lo + BLK],
                                             t1[:], t2[:])
                nc.sync.dma_start(
                    qT_d[b * NH_LOC:(b + 1) * NH_LOC, :, lo:lo + BLK]
                    .rearrange("h p t -> p h t"), qr[:])

                vsb = p1t.tile([P, BLK // P, D_LOC], F32R, name="vsb")
                for tt in range(BLK // P):
                    psv = ps1v.tile([P, D_LOC], F32, name="vps")
                    for kc in range(KC):
                        nc.tensor.matmul(
                            psv[:], lhsT=xb[:, kc, tt * P:(tt + 1) * P],
                            rhs=wt[:, kc, 2 * D_LOC:3 * D_LOC],
                            start=(kc == 0), stop=(kc == KC - 1))
                    nc.vector.tensor_copy(vsb[:, tt, :], psv[:])
                # one DMA per head covering both token tiles of this block
                for hh in range(NH_LOC):
                    nc.sync.dma_start(
                        vN_d[b * NH_LOC + hh, lo:lo + BLK, :]
                        .rearrange("(n p) d -> p n d", p=P),
                        vsb[:, :, hh * HD:(hh + 1) * HD])

        # ---------------- phase 2: causal attention ----------------
        trimask = p2c.tile([P, P], F32)
        nc.sync.dma_start(trimask[:], tri[:])
        ones = p2c.tile([P, P], F32R)
        nc.sync.dma_start(ones[:], ones_in[:])

        for bh in range(NBH):
            qt = p2q.tile([P, L], F32R, name="qt")
            nc.sync.dma_start(qt[:], qT_d[bh])
            vn = p2v.tile([P, KT, HD], F32R, name="vn")
            nc.sync.dma_start(vn[:],
                              vN_d[bh].rearrange("(n p) d -> p n d", p=P))
            for qs_i in range(NQS):
                qs = qs_i * QS
                nkt = (qs + QS) // P
                po = ps2o.tile([P, QS], F32, name="po")
                pd = ps2d.tile([P, QS], F32, name="pd")
                for k_i in range(nkt):
                    d = k_i * P - qs
                    c0 = max(d, 0)
                    psc = ps2s.tile([P, QS], F32, name="psc")
                    nc.tensor.matmul(
                        psc[:, c0:QS],
                        lhsT=kt_all[:, bh, k_i * P:(k_i + 1) * P],
                        rhs=qt[:, qs + c0:qs + QS],
                        start=True, stop=True)
                    et = p2e.tile([P, QS], F32R, name="et")
                    if d >= 0:
                        smsm = p2t.tile([P, P], F32, name="smsm")
                        nc.vector.tensor_add(smsm[:], psc[:, d:d + P],
                                             trimask[:])
                        nc.scalar.activation(et[:, d:d + P], smsm[:], EXP)
                        if d + P < QS:
                            nc.scalar.activation(et[:, d + P:QS],
                                                 psc[:, d + P:QS], EXP)
                    else:
                        nc.scalar.activation(et[:, 0:QS], psc[:, 0:QS], EXP)
                    nc.tensor.matmul(po[:, c0:QS], lhsT=vn[:, k_i, :],
                                     rhs=et[:, c0:QS], start=(k_i == 0),
                                     stop=(k_i == nkt - 1))
                    nc.tensor.matmul(pd[:, c0:QS], lhsT=ones[:],
                                     rhs=et[:, c0:QS], start=(k_i == 0),
                                     stop=(k_i == nkt - 1))
                rec = p2t.tile([P, QS], F32, name="rec")
                nc.vector.reciprocal(rec[:], pd[:])
                ot = p2t.tile([P, QS], F32R, name="ot")
                nc.vector.tensor_mul(ot[:], po[:], rec[:])
                nc.sync.dma_start(oT_d[bh, :, qs:qs + QS], ot[:])

        # ---------------- phase 3: output projection (partial) ----------------
        with tc.tile_pool(name="p3w", bufs=1) as p3w, \
             tc.tile_pool(name="p3b", bufs=2) as p3b, \
             tc.tile_pool(name="p3y", bufs=3) as p3y, \
             tc.tile_pool(name="ps3", bufs=2, space="PSUM") as ps3:
            wo = p3w.tile([P, NH_LOC, H], F32R)
            for hh in range(NH_LOC):
                nc.sync.dma_start(wo[:, hh, :], woT[hh * P:(hh + 1) * P, :])
            for b in range(B):
                otb = p3b.tile([P, NH_LOC, L], F32R, name="otb")
                for hh in range(NH_LOC):
                    for qi in range(NQS):
                        sl = slice(qi * QS, (qi + 1) * QS)
                        nc.sync.dma_start(otb[:, hh, sl],
                                          oT_d[b * NH_LOC + hh][:, sl])
                for tt in range(L // P):
                    ybig = p3y.tile([P, H], F32, name="ybig")
                    for oc in range(H // 512):
                        py_ = ps3.tile([P, 512], F32, name="py")
                        for hh in range(NH_LOC):
                            nc.tensor.matmul(
                                py_[:],
                                lhsT=otb[:, hh, tt * P:(tt + 1) * P],
                                rhs=wo[:, hh, oc * 512:(oc + 1) * 512],
                                start=(hh == 0), stop=(hh == NH_LOC - 1))
                        nc.vector.tensor_copy(
                            ybig[:, oc * 512:(oc + 1) * 512], py_[:])
                    nc.sync.dma_start(
                        y[b * L + tt * P: b * L + (tt + 1) * P, :], ybig[:])

    nc.compile()
    return nc


_NC = None


def _get_nc():
    global _NC
    if _NC is None:
        _NC = _build()
    return _NC


def _host_inputs(x, Wqkv, Wo):
    x = np.asarray(x, dtype=np.float32)
    Wqkv = np.asarray(Wqkv, dtype=np.float32)
    Wo = np.asarray(Wo, dtype=np.float32)

    xT = np.ascontiguousarray(x.reshape(BL, H).T)

    inv_freq = 1.0 / (ROPE_THETA ** (np.arange(0, HD, 2, dtype=np.float32)
                                     / HD))
    t = np.arange(L, dtype=np.float32)
    freqs = np.outer(t, inv_freq).astype(np.float32)
    emb = np.concatenate([freqs, freqs], axis=-1)
    cosT = np.ascontiguousarray(np.cos(emb).T.astype(np.float32))
    sinT = np.sin(emb).T.astype(np.float32)
    sinTs = np.ascontiguousarray(np.concatenate([-sinT[:64], sinT[64:]], 0))

    kk = np.arange(P)[:, None]
    qq = np.arange(P)[None, :]
    tri = np.where(qq >= kk, 0.0, NEG).astype(np.float32)

    scale = np.float32(1.0 / math.sqrt(HD))
    in_maps = []
    for c in range(N_CORES):
        r0 = c * D_LOC
        wq = Wqkv[r0:r0 + D_LOC] * scale
        wk = Wqkv[H + r0:H + r0 + D_LOC]
        wv = Wqkv[2 * H + r0:2 * H + r0 + D_LOC]
        wT_c = np.ascontiguousarray(np.concatenate([wq, wk, wv], 0).T)
        woT_c = np.ascontiguousarray(Wo[:, r0:r0 + D_LOC].T)
        in_maps.append({
            "xT": xT, "wT": wT_c, "woT": woT_c,
            "cosT": cosT, "sinTs": sinTs, "tri": tri,
            "ones": np.ones((P, P), dtype=np.float32),
        })
    return in_maps


def kernel(x, Wqkv, Wo):
    nc = _get_nc()
    in_maps = _host_inputs(x, Wqkv, Wo)
    res = run_bass_kernel_spmd(nc, in_maps, list(range(N_CORES)))
    y = res.results[0]["y"].astype(np.float64)
    for c in range(1, N_CORES):
        y += res.results[c]["y"]
    return y.astype(np.float32).reshape(B, L, H)
